# revision 18
# baseline (speedup 1.0000x reference)
"""Trainium2 Bass kernel for nn_AdaptiveExpertSystem (MoE, E=8, top-2).

Expert-parallel + pairwise tensor-parallel design. The host computes the
(cheap) router on CPU and uses it as the sharding function. Experts are
paired heavy+light (LPT), one pair per pair of cores: both cores of a
pair receive ALL tokens routed to either expert (pre-normalized by the
LN the router already computed, pre-transposed to feature-major), but
each core holds only HALF of the pair's w1/w2 along the intermediate
dimension I. Each core computes partial MLP outputs for all the pair's
tokens at the PE bf16 matmul roofline; the host sums the two halves and
scatter-adds with the top-2 combine weights (the unshard step).

ln_g/ln_b are folded into w1/b1 on the host so all experts share the
plain LN.

v2 (DMA restructure): the PE is the bottleneck (one continuous matmul
block at ~0.505 ns/col); the rest is head/tail engineering.
 - Every token chunk gets its OWN contiguous SBUF tile, so token DMAs
   have multi-KB per-partition runs (fast big-line path) instead of
   strided ~1KB writes into one wide tile (slow ~24 GB/s/queue path).
 - chunk0 and the final chunk are narrowed to 256 columns: chunk0's
   tokens gate the first matmul; the final chunk's output gates the
   kernel end.
 - Outputs are written per-chunk in [128, ND, w] layout and DMAed in
   d-groups of 4 (2 for the last chunk) -> 2-4 KB runs, alternating
   the sync/scalar queues.
 - Bulk weights ride gpsimd in consumption order; token chunks
   round-robin sync/scalar.
"""
import numpy as np
import ml_dtypes

import concourse.bass as bass
import concourse.tile as tile
from concourse import bacc, mybir
from concourse.bass_utils import run_bass_kernel_spmd

N_CORES = 8
B, L, D, I, E = 2, 2048, 1024, 4096, 8
NTOK = B * L
KD = D // 128       # 8  d-tiles (contraction of mm1)
NI = I // 128       # 32 i-tiles total; each core runs NIH = 16
NIH = NI // 2
ND = D // 128       # 8  output d-tiles
LN_EPS = 1e-5

F32 = mybir.dt.float32
BF16 = mybir.dt.bfloat16
BF = ml_dtypes.bfloat16

_CACHE = {}


def _chunks_one(n, first, last):
    # chunk widths for one expert's n columns; every chunk in [288, 508]
    # (>=288 keeps the ~117ns LDWEIGHTS hidden behind the previous
    # matmul at 0.425 ns/col, <=508 fits a PSUM bank). slot0 leads with
    # a 384 chunk: narrow enough that its token DMA doesn't gate the
    # first matmul for long, wide enough that its weight-tile demand
    # (256KB per 8*w cols) stays under the gpsimd supply rate. slot1
    # ends with a narrow 288 chunk (less output DMA after the last
    # matmul).
    ws = []
    rest = n
    head = tail = 0
    if first and n > 892:
        head = 384
        rest -= 384
    if last and rest > 796:
        tail = 288
        rest -= 288
    nch = max(1, (rest + 507) // 508)
    base = rest // nch
    extra = rest - base * nch
    if head:
        ws.append(head)
    for j in range(nch):
        ws.append(base + (1 if j < extra else 0))
    if tail:
        ws.append(tail)
    return ws


def _chunk_table(n1, n2):
    """[(expert_slot, col_offset, width), ...] covering [0,n1)+[n1,n1+n2)."""
    out = []
    o = 0
    for sl, n in ((0, n1), (1, n2)):
        for w in _chunks_one(n, first=(sl == 0), last=(sl == 1)):
            out.append((sl, o, w))
            o += w
    return out


def build_nc(n1, n2):
    chunks = _chunk_table(n1, n2)

    nc = bacc.Bacc(None, num_devices=N_CORES)
    xt_ps = [nc.declare_dram_parameter(f"xt{j}", [128, KD, w], BF16,
                                       isOutput=False)
             for j, (_, _, w) in enumerate(chunks)]
    # slot0 weights as per-tile arrays (need-ordered individual DMAs
    # during the head crunch); slot1 weights partition-major so each
    # transfers as ONE big DMA (32KB/partition contiguous runs)
    w1_p = nc.declare_dram_parameter("w1", [NIH, 128, KD, 128], BF16,
                                     isOutput=False)
    w2_p = nc.declare_dram_parameter("w2", [ND, 128, NIH, 128], BF16,
                                     isOutput=False)
    w1b_p = nc.declare_dram_parameter("w1b", [128, NIH, KD, 128], BF16,
                                      isOutput=False)
    w2b_p = nc.declare_dram_parameter("w2b", [128, ND, NIH, 128], BF16,
                                      isOutput=False)
    b1_p = nc.declare_dram_parameter("b1", [128, 2, NIH], F32, isOutput=False)
    out_ps = [nc.declare_dram_parameter(f"out{j}", [128, ND, w], BF16,
                                        isOutput=True)
              for j, (_, _, w) in enumerate(chunks)]

    AF = mybir.ActivationFunctionType

    from contextlib import ExitStack
    with tile.TileContext(nc) as tc, ExitStack() as ctx:
        ep = ctx.enter_context
        xtp = ep(tc.tile_pool(name="xtp", bufs=1))
        w1pool = ep(tc.tile_pool(name="w1p", bufs=1))
        w2pool = ep(tc.tile_pool(name="w2p", bufs=1))
        b1pool = ep(tc.tile_pool(name="b1p", bufs=1))
        h1pool = ep(tc.tile_pool(name="h1p", bufs=1))
        h2pool = ep(tc.tile_pool(name="h2p", bufs=2))
        ps1 = ep(tc.tile_pool(name="ps1", bufs=4, space="PSUM"))
        ps2 = ep(tc.tile_pool(name="ps2", bufs=2, space="PSUM"))

        b1sb = b1pool.tile([128, 2, NIH], F32)
        nc.scalar.dma_start(out=b1sb, in_=b1_p[:])
        w1sb = w1pool.tile([128, 2, NIH, KD, 128], BF16)
        w2sb = w2pool.tile([128, 2, ND, NIH, 128], BF16)
        # one contiguous SBUF tile per token chunk: per-partition runs of
        # KD*w*2 bytes make these DMAs ride the fast big-line path
        xsb = [xtp.tile([128, KD, w], BF16, name=f"xsb{j}")
               for j, (_, _, w) in enumerate(chunks)]

        # The scalar (Activation) engine must carry NO bulk DMA issue:
        # each HWDGE ring holds only ~2 outstanding transfers, so a
        # DMA_DIRECT2D on scalar blocks until older transfers drain —
        # which postpones the GELU ACTIVATEs, which stalls the PE on
        # PSUM-bank reuse (matmul i needs activation i-4 done). So:
        # scalar gets only the tiny b1; sync + gpsimd split everything
        # else in need order. The first-matmul gate is chunk0's tokens
        # + w1[0,0]; chunk0's mm1 sweeps all 4MB of w1[slot0] at
        # ~190GB/s demand vs ~122+135 GB/s combined queue rate, so
        # slot0 w1 tiles alternate sync/gpsimd by need index.
        nc.sync.dma_start(out=xsb[0][:, 0:KD // 2],
                          in_=xt_ps[0][:, 0:KD // 2, :])
        nc.gpsimd.dma_start(out=w1sb[:, 0, 0], in_=w1_p[0])
        nc.gpsimd.dma_start(out=xsb[0][:, KD // 2:],
                            in_=xt_ps[0][:, KD // 2:, :])
        for i in range(1, NIH):
            q = nc.sync if i % 2 else nc.gpsimd
            q.dma_start(out=w1sb[:, 0, i], in_=w1_p[i])
        for d in range(ND):
            q = nc.gpsimd if d % 2 == 0 else nc.sync
            q.dma_start(out=w2sb[:, 0, d], in_=w2_p[d])
        # remaining token chunks on sync (first needed at ~55us, they
        # queue behind the slot0 weights); slot1 weights on gpsimd as
        # two single big DMAs (consumed only from ~halfway through =>
        # plenty of slack)
        for j in range(1, len(chunks)):
            nc.sync.dma_start(out=xsb[j], in_=xt_ps[j][:])
        for i in range(NIH):
            nc.gpsimd.dma_start(out=w1sb[:, 1, i], in_=w1b_p[:, i])
        for d in range(ND):
            nc.gpsimd.dma_start(out=w2sb[:, 1, d], in_=w2b_p[:, d])

        out_qs = [nc.sync, nc.gpsimd]
        _oq = [0]

        def out_dma(dst, src):
            q = out_qs[_oq[0] % 2]
            _oq[0] += 1
            q.dma_start(out=dst, in_=src)

        for ci, (sl, c0, w) in enumerate(chunks):
            # ---- mm1 + gelu -> h1 (this chunk, this expert slot) ----
            h1 = h1pool.tile([128, NIH, w], BF16, name=f"h1_{ci}", tag="h1",
                             bufs=1)
            for i in range(NIH):
                p1 = ps1.tile([128, w], F32, tag="p1", name=f"p1_{ci}_{i}")
                for k in range(KD):
                    nc.tensor.matmul(p1, lhsT=w1sb[:, sl, i, k],
                                     rhs=xsb[ci][:, k],
                                     start=(k == 0), stop=(k == KD - 1))
                nc.scalar.activation(out=h1[:, i], in_=p1, func=AF.Gelu,
                                     bias=b1sb[:, sl, i:i + 1], scale=1.0)
            # ---- mm2 -> partial h2 -> grouped DMA out ----
            # d-groups of 4 (2 on the final chunk) give 2-4KB DRAM runs
            last = (ci == len(chunks) - 1)
            G = 2 if last else 4
            for d0 in range(0, ND, G):
                h2 = h2pool.tile([128, G, w], BF16, tag="h2",
                                 name=f"h2_{ci}_{d0}")
                for dd in range(0, G, 2):
                    # p2 spans TWO psum banks (padded to the 2KB bank
                    # stride) so one CAST covers a d-pair
                    p2 = ps2.tile([128, 2, w], F32, tag="p2",
                                  name=f"p2_{ci}_{d0 + dd}",
                                  padded_shape=[128, 2, 512])
                    for g in range(2):
                        d = d0 + dd + g
                        for i in range(NIH):
                            nc.tensor.matmul(p2[:, g], lhsT=w2sb[:, sl, d, i],
                                             rhs=h1[:, i],
                                             start=(i == 0),
                                             stop=(i == NIH - 1))
                    nc.vector.tensor_copy(out=h2[:, dd:dd + 2], in_=p2)
                out_dma(out_ps[ci][:, d0:d0 + G, :], h2)

    nc.finalize()
    return nc


def _pack_w1h(w1e_half):
    # [d, i_half] -> [i_tile, p, k, m]; d = k*128 + p, i = it*128 + m
    t = w1e_half.reshape(KD, 128, NIH, 128)
    return np.ascontiguousarray(t.transpose(2, 1, 0, 3)).astype(BF)


def _pack_w2h(w2e_half):
    # [i_half, d] -> [d_tile, p, i_tile, m]; i = it*128 + p, d = dt*128 + m
    t = w2e_half.reshape(NIH, 128, ND, 128)
    return np.ascontiguousarray(t.transpose(2, 1, 0, 3)).astype(BF)


def kernel(**inputs) -> np.ndarray:
    x = np.asarray(inputs["hidden_states"], np.float32).reshape(NTOK, D)
    rn_g = np.asarray(inputs["rn_g"], np.float32)
    rn_b = np.asarray(inputs["rn_b"], np.float32)
    router_w = np.asarray(inputs["router_w"], np.float32)
    router_b = np.asarray(inputs["router_b"], np.float32)
    ln_g = np.asarray(inputs["ln_g"], np.float32)
    ln_b = np.asarray(inputs["ln_b"], np.float32)
    w1 = np.asarray(inputs["w1"], np.float32)
    b1 = np.asarray(inputs["b1"], np.float32)
    w2 = np.asarray(inputs["w2"], np.float32)
    b2 = np.asarray(inputs["b2"], np.float32)

    # ---- Router on host: this IS the sharding function ----
    m = x.mean(-1, keepdims=True)
    v = ((x - m) ** 2).mean(-1, keepdims=True)
    rstd = 1.0 / np.sqrt(v + LN_EPS)
    normed = (x - m) * rstd
    logits = (normed * rn_g + rn_b) @ router_w.T + router_b
    top2 = np.argsort(-logits, axis=-1, kind="stable")[:, :2]
    tv = np.take_along_axis(logits, top2, -1)
    tv = np.exp(tv - tv.max(-1, keepdims=True))
    tw = (tv / tv.sum(-1, keepdims=True)).astype(np.float32)

    idxs, wts = [], []
    for e in range(E):
        sel = (top2[:, 0] == e) | (top2[:, 1] == e)
        idx_e = np.nonzero(sel)[0]
        w_e = np.where(top2[idx_e, 0] == e, tw[idx_e, 0], tw[idx_e, 1])
        idxs.append(idx_e)
        wts.append(w_e.astype(np.float32))

    # LPT pairing: heaviest with lightest
    order = sorted(range(E), key=lambda e: -len(idxs[e]))
    pairs = [(order[i], order[E - 1 - i]) for i in range(E // 2)]
    # round counts to multiples of 4 for clean strides
    caps = {e: max(256, ((len(idxs[e]) + 3) // 4) * 4) for e in range(E)}
    n1g = max(caps[a] for a, b in pairs)
    n2g = max(caps[b] for a, b in pairs)

    normed_bf = normed.astype(BF)
    in_maps = []
    chunks = _chunk_table(n1g, n2g)
    for a, b_ in pairs:
        g = np.zeros((n1g + n2g, D), dtype=BF)
        g[:len(idxs[a])] = normed_bf[idxs[a]]
        g[n1g:n1g + len(idxs[b_])] = normed_bf[idxs[b_]]
        t = g.T.reshape(KD, 128, n1g + n2g).transpose(1, 0, 2)
        xts = {}
        for j, (sl, c0, w) in enumerate(chunks):
            xts[f"xt{j}"] = np.ascontiguousarray(t[:, :, c0:c0 + w])
        for half in range(2):
            ilo, ihi = half * (I // 2), (half + 1) * (I // 2)
            w1sl = [_pack_w1h((ln_g[e][:, None] * w1[e])[:, ilo:ihi])
                    for e in (a, b_)]
            w2sl = [_pack_w2h(w2[e][ilo:ihi, :]) for e in (a, b_)]
            b1s = np.stack(
                [(b1[e] + ln_b[e] @ w1[e])[ilo:ihi].reshape(NIH, 128).T
                 .astype(np.float32) for e in (a, b_)], axis=1)
            mp = {"w1": w1sl[0], "w2": w2sl[0],
                  "w1b": np.ascontiguousarray(w1sl[1].transpose(1, 0, 2, 3)),
                  "w2b": np.ascontiguousarray(w2sl[1].transpose(1, 0, 2, 3)),
                  "b1": np.ascontiguousarray(b1s)}
            mp.update(xts)
            in_maps.append(mp)

    key = (n1g, n2g)
    if key not in _CACHE:
        _CACHE[key] = build_nc(n1g, n2g)
    nc = _CACHE[key]
    res = run_bass_kernel_spmd(nc, in_maps, core_ids=list(range(N_CORES)))

    # ---- Unshard: sum I-halves, weighted scatter-add (top-2 combine) ----
    out = tw[:, 0:1] * b2[top2[:, 0]] + tw[:, 1:2] * b2[top2[:, 1]]
    for pi, (a, b_) in enumerate(pairs):
        ra = res.results[2 * pi]
        rb = res.results[2 * pi + 1]
        # out{j}: [128, ND, w] -> [D, w]; concat chunks -> [D, cap] -> .T
        cols = []
        for j in range(len(chunks)):
            h2j = (np.asarray(ra[f"out{j}"], np.float32)
                   + np.asarray(rb[f"out{j}"], np.float32))
            cols.append(h2j.transpose(1, 0, 2).reshape(D, -1))
        h2 = np.concatenate(cols, axis=1).T     # [cap, D]
        out[idxs[a]] += wts[a][:, None] * h2[:len(idxs[a])]
        out[idxs[b_]] += wts[b_][:, None] * h2[n1g:n1g + len(idxs[b_])]
    return out.reshape(B, L, D).astype(np.float32)


# revision 19
# speedup vs baseline: 1.0062x; 1.0062x over previous
"""Trainium2 Bass kernel for nn_AdaptiveExpertSystem (MoE, E=8, top-2).

Expert-parallel + pairwise tensor-parallel design. The host computes the
(cheap) router on CPU and uses it as the sharding function. Experts are
paired heavy+light (LPT), one pair per pair of cores: both cores of a
pair receive ALL tokens routed to either expert (pre-normalized by the
LN the router already computed, pre-transposed to feature-major), but
each core holds only HALF of the pair's w1/w2 along the intermediate
dimension I. Each core computes partial MLP outputs for all the pair's
tokens at the PE bf16 matmul roofline; the host sums the two halves and
scatter-adds with the top-2 combine weights (the unshard step).

ln_g/ln_b are folded into w1/b1 on the host so all experts share the
plain LN.

v2 (DMA restructure): the PE is the bottleneck (one continuous matmul
block at ~0.505 ns/col); the rest is head/tail engineering.
 - Every token chunk gets its OWN contiguous SBUF tile, so token DMAs
   have multi-KB per-partition runs (fast big-line path) instead of
   strided ~1KB writes into one wide tile (slow ~24 GB/s/queue path).
 - chunk0 and the final chunk are narrowed to 256 columns: chunk0's
   tokens gate the first matmul; the final chunk's output gates the
   kernel end.
 - Outputs are written per-chunk in [128, ND, w] layout and DMAed in
   d-groups of 4 (2 for the last chunk) -> 2-4 KB runs, alternating
   the sync/scalar queues.
 - Bulk weights ride gpsimd in consumption order; token chunks
   round-robin sync/scalar.
"""
import numpy as np
import ml_dtypes

import concourse.bass as bass
import concourse.tile as tile
from concourse import bacc, mybir
from concourse.bass_utils import run_bass_kernel_spmd

N_CORES = 8
B, L, D, I, E = 2, 2048, 1024, 4096, 8
NTOK = B * L
KD = D // 128       # 8  d-tiles (contraction of mm1)
NI = I // 128       # 32 i-tiles total; each core runs NIH = 16
NIH = NI // 2
ND = D // 128       # 8  output d-tiles
LN_EPS = 1e-5

F32 = mybir.dt.float32
BF16 = mybir.dt.bfloat16
BF = ml_dtypes.bfloat16

_CACHE = {}


def _chunks_one(n, first, last):
    # chunk widths for one expert's n columns; every chunk in [288, 508]
    # (>=288 keeps the ~117ns LDWEIGHTS hidden behind the previous
    # matmul at 0.425 ns/col, <=508 fits a PSUM bank). slot0 leads with
    # a 384 chunk: narrow enough that its token DMA doesn't gate the
    # first matmul for long, wide enough that its weight-tile demand
    # (256KB per 8*w cols) stays under the gpsimd supply rate. slot1
    # ends with a narrow 288 chunk (less output DMA after the last
    # matmul).
    ws = []
    rest = n
    head = tail = 0
    if first and n > 892:
        head = 384
        rest -= 384
    if last and rest > 796:
        tail = 288
        rest -= 288
    nch = max(1, (rest + 507) // 508)
    base = rest // nch
    extra = rest - base * nch
    if head:
        ws.append(head)
    for j in range(nch):
        ws.append(base + (1 if j < extra else 0))
    if tail:
        ws.append(tail)
    return ws


def _chunk_table(n1, n2):
    """[(expert_slot, col_offset, width), ...] covering [0,n1)+[n1,n1+n2)."""
    out = []
    o = 0
    for sl, n in ((0, n1), (1, n2)):
        for w in _chunks_one(n, first=(sl == 0), last=(sl == 1)):
            out.append((sl, o, w))
            o += w
    return out


def build_nc(n1, n2):
    chunks = _chunk_table(n1, n2)

    nc = bacc.Bacc(None, num_devices=N_CORES)
    xt_ps = [nc.declare_dram_parameter(f"xt{j}", [128, KD, w], BF16,
                                       isOutput=False)
             for j, (_, _, w) in enumerate(chunks)]
    # slot0 weights as per-tile arrays (need-ordered individual DMAs
    # during the head crunch); slot1 weights partition-major so each
    # transfers as ONE big DMA (32KB/partition contiguous runs)
    w1_p = nc.declare_dram_parameter("w1", [NIH, 128, KD, 128], BF16,
                                     isOutput=False)
    w2_p = nc.declare_dram_parameter("w2", [ND, 128, NIH, 128], BF16,
                                     isOutput=False)
    w1b_p = nc.declare_dram_parameter("w1b", [128, NIH, KD, 128], BF16,
                                      isOutput=False)
    w2b_p = nc.declare_dram_parameter("w2b", [128, ND, NIH, 128], BF16,
                                      isOutput=False)
    b1_p = nc.declare_dram_parameter("b1", [128, 2, NIH], F32, isOutput=False)
    out_ps = [nc.declare_dram_parameter(f"out{j}", [128, ND, w], BF16,
                                        isOutput=True)
              for j, (_, _, w) in enumerate(chunks)]

    AF = mybir.ActivationFunctionType

    from contextlib import ExitStack
    with tile.TileContext(nc) as tc, ExitStack() as ctx:
        ep = ctx.enter_context
        xtp = ep(tc.tile_pool(name="xtp", bufs=1))
        w1pool = ep(tc.tile_pool(name="w1p", bufs=1))
        w2pool = ep(tc.tile_pool(name="w2p", bufs=1))
        b1pool = ep(tc.tile_pool(name="b1p", bufs=1))
        h1pool = ep(tc.tile_pool(name="h1p", bufs=1))
        h2pool = ep(tc.tile_pool(name="h2p", bufs=2))
        ps1 = ep(tc.tile_pool(name="ps1", bufs=4, space="PSUM"))
        ps2 = ep(tc.tile_pool(name="ps2", bufs=2, space="PSUM"))

        b1sb = b1pool.tile([128, 2, NIH], F32)
        nc.scalar.dma_start(out=b1sb, in_=b1_p[:])
        w1sb = w1pool.tile([128, 2, NIH, KD, 128], BF16)
        w2sb = w2pool.tile([128, 2, ND, NIH, 128], BF16)
        # one contiguous SBUF tile per token chunk: per-partition runs of
        # KD*w*2 bytes make these DMAs ride the fast big-line path
        xsb = [xtp.tile([128, KD, w], BF16, name=f"xsb{j}")
               for j, (_, _, w) in enumerate(chunks)]

        # The scalar (Activation) engine must carry NO bulk DMA issue:
        # each HWDGE ring holds only ~2 outstanding transfers, so a
        # DMA_DIRECT2D on scalar blocks until older transfers drain —
        # which postpones the GELU ACTIVATEs, which stalls the PE on
        # PSUM-bank reuse (matmul i needs activation i-4 done). So:
        # scalar gets only the tiny b1; sync + gpsimd split everything
        # else in need order. The first-matmul gate is chunk0's tokens
        # + w1[0,0]; chunk0's mm1 sweeps all 4MB of w1[slot0] at
        # ~190GB/s demand vs ~122+135 GB/s combined queue rate, so
        # slot0 w1 tiles alternate sync/gpsimd by need index.
        nc.sync.dma_start(out=xsb[0][:, 0:KD // 2],
                          in_=xt_ps[0][:, 0:KD // 2, :])
        nc.gpsimd.dma_start(out=w1sb[:, 0, 0], in_=w1_p[0])
        # chunk0's upper k-half split scalar/gpsimd: scalar can afford
        # exactly ONE early bulk transfer before its ACTIVATEs start
        nc.scalar.dma_start(out=xsb[0][:, KD // 2:KD // 2 + 2],
                            in_=xt_ps[0][:, KD // 2:KD // 2 + 2, :])
        nc.gpsimd.dma_start(out=xsb[0][:, KD // 2 + 2:],
                            in_=xt_ps[0][:, KD // 2 + 2:, :])
        for i in range(1, NIH):
            q = nc.sync if i % 2 else nc.gpsimd
            q.dma_start(out=w1sb[:, 0, i], in_=w1_p[i])
        for d in range(ND):
            q = nc.gpsimd if d % 2 == 0 else nc.sync
            q.dma_start(out=w2sb[:, 0, d], in_=w2_p[d])
        # remaining token chunks on sync (first needed at ~55us, they
        # queue behind the slot0 weights); slot1 weights on gpsimd as
        # two single big DMAs (consumed only from ~halfway through =>
        # plenty of slack)
        for j in range(1, len(chunks)):
            nc.sync.dma_start(out=xsb[j], in_=xt_ps[j][:])
        for i in range(NIH):
            nc.gpsimd.dma_start(out=w1sb[:, 1, i], in_=w1b_p[:, i])
        for d in range(ND):
            nc.gpsimd.dma_start(out=w2sb[:, 1, d], in_=w2b_p[:, d])

        out_qs = [nc.sync, nc.gpsimd]
        _oq = [0]

        def out_dma(dst, src):
            q = out_qs[_oq[0] % 2]
            _oq[0] += 1
            q.dma_start(out=dst, in_=src)

        for ci, (sl, c0, w) in enumerate(chunks):
            # ---- mm1 + gelu -> h1 (this chunk, this expert slot) ----
            h1 = h1pool.tile([128, NIH, w], BF16, name=f"h1_{ci}", tag="h1",
                             bufs=1)
            for i in range(NIH):
                p1 = ps1.tile([128, w], F32, tag="p1", name=f"p1_{ci}_{i}")
                for k in range(KD):
                    nc.tensor.matmul(p1, lhsT=w1sb[:, sl, i, k],
                                     rhs=xsb[ci][:, k],
                                     start=(k == 0), stop=(k == KD - 1))
                nc.scalar.activation(out=h1[:, i], in_=p1, func=AF.Gelu,
                                     bias=b1sb[:, sl, i:i + 1], scale=1.0)
            # ---- mm2 -> partial h2 -> grouped DMA out ----
            # d-groups of 4 (2 on the final chunk) give 2-4KB DRAM runs
            last = (ci == len(chunks) - 1)
            G = 2 if last else 4
            for d0 in range(0, ND, G):
                h2 = h2pool.tile([128, G, w], BF16, tag="h2",
                                 name=f"h2_{ci}_{d0}")
                for dd in range(0, G, 2):
                    # p2 spans TWO psum banks (padded to the 2KB bank
                    # stride) so one CAST covers a d-pair
                    p2 = ps2.tile([128, 2, w], F32, tag="p2",
                                  name=f"p2_{ci}_{d0 + dd}",
                                  padded_shape=[128, 2, 512])
                    for g in range(2):
                        d = d0 + dd + g
                        for i in range(NIH):
                            nc.tensor.matmul(p2[:, g], lhsT=w2sb[:, sl, d, i],
                                             rhs=h1[:, i],
                                             start=(i == 0),
                                             stop=(i == NIH - 1))
                    nc.vector.tensor_copy(out=h2[:, dd:dd + 2], in_=p2)
                out_dma(out_ps[ci][:, d0:d0 + G, :], h2)

    nc.finalize()
    return nc


def _pack_w1h(w1e_half):
    # [d, i_half] -> [i_tile, p, k, m]; d = k*128 + p, i = it*128 + m
    t = w1e_half.reshape(KD, 128, NIH, 128)
    return np.ascontiguousarray(t.transpose(2, 1, 0, 3)).astype(BF)


def _pack_w2h(w2e_half):
    # [i_half, d] -> [d_tile, p, i_tile, m]; i = it*128 + p, d = dt*128 + m
    t = w2e_half.reshape(NIH, 128, ND, 128)
    return np.ascontiguousarray(t.transpose(2, 1, 0, 3)).astype(BF)


def kernel(**inputs) -> np.ndarray:
    x = np.asarray(inputs["hidden_states"], np.float32).reshape(NTOK, D)
    rn_g = np.asarray(inputs["rn_g"], np.float32)
    rn_b = np.asarray(inputs["rn_b"], np.float32)
    router_w = np.asarray(inputs["router_w"], np.float32)
    router_b = np.asarray(inputs["router_b"], np.float32)
    ln_g = np.asarray(inputs["ln_g"], np.float32)
    ln_b = np.asarray(inputs["ln_b"], np.float32)
    w1 = np.asarray(inputs["w1"], np.float32)
    b1 = np.asarray(inputs["b1"], np.float32)
    w2 = np.asarray(inputs["w2"], np.float32)
    b2 = np.asarray(inputs["b2"], np.float32)

    # ---- Router on host: this IS the sharding function ----
    m = x.mean(-1, keepdims=True)
    v = ((x - m) ** 2).mean(-1, keepdims=True)
    rstd = 1.0 / np.sqrt(v + LN_EPS)
    normed = (x - m) * rstd
    logits = (normed * rn_g + rn_b) @ router_w.T + router_b
    top2 = np.argsort(-logits, axis=-1, kind="stable")[:, :2]
    tv = np.take_along_axis(logits, top2, -1)
    tv = np.exp(tv - tv.max(-1, keepdims=True))
    tw = (tv / tv.sum(-1, keepdims=True)).astype(np.float32)

    idxs, wts = [], []
    for e in range(E):
        sel = (top2[:, 0] == e) | (top2[:, 1] == e)
        idx_e = np.nonzero(sel)[0]
        w_e = np.where(top2[idx_e, 0] == e, tw[idx_e, 0], tw[idx_e, 1])
        idxs.append(idx_e)
        wts.append(w_e.astype(np.float32))

    # LPT pairing: heaviest with lightest
    order = sorted(range(E), key=lambda e: -len(idxs[e]))
    pairs = [(order[i], order[E - 1 - i]) for i in range(E // 2)]
    # round counts to multiples of 4 for clean strides
    caps = {e: max(256, ((len(idxs[e]) + 3) // 4) * 4) for e in range(E)}
    n1g = max(caps[a] for a, b in pairs)
    n2g = max(caps[b] for a, b in pairs)

    normed_bf = normed.astype(BF)
    in_maps = []
    chunks = _chunk_table(n1g, n2g)
    for a, b_ in pairs:
        g = np.zeros((n1g + n2g, D), dtype=BF)
        g[:len(idxs[a])] = normed_bf[idxs[a]]
        g[n1g:n1g + len(idxs[b_])] = normed_bf[idxs[b_]]
        t = g.T.reshape(KD, 128, n1g + n2g).transpose(1, 0, 2)
        xts = {}
        for j, (sl, c0, w) in enumerate(chunks):
            xts[f"xt{j}"] = np.ascontiguousarray(t[:, :, c0:c0 + w])
        for half in range(2):
            ilo, ihi = half * (I // 2), (half + 1) * (I // 2)
            w1sl = [_pack_w1h((ln_g[e][:, None] * w1[e])[:, ilo:ihi])
                    for e in (a, b_)]
            w2sl = [_pack_w2h(w2[e][ilo:ihi, :]) for e in (a, b_)]
            b1s = np.stack(
                [(b1[e] + ln_b[e] @ w1[e])[ilo:ihi].reshape(NIH, 128).T
                 .astype(np.float32) for e in (a, b_)], axis=1)
            mp = {"w1": w1sl[0], "w2": w2sl[0],
                  "w1b": np.ascontiguousarray(w1sl[1].transpose(1, 0, 2, 3)),
                  "w2b": np.ascontiguousarray(w2sl[1].transpose(1, 0, 2, 3)),
                  "b1": np.ascontiguousarray(b1s)}
            mp.update(xts)
            in_maps.append(mp)

    key = (n1g, n2g)
    if key not in _CACHE:
        _CACHE[key] = build_nc(n1g, n2g)
    nc = _CACHE[key]
    res = run_bass_kernel_spmd(nc, in_maps, core_ids=list(range(N_CORES)))

    # ---- Unshard: sum I-halves, weighted scatter-add (top-2 combine) ----
    out = tw[:, 0:1] * b2[top2[:, 0]] + tw[:, 1:2] * b2[top2[:, 1]]
    for pi, (a, b_) in enumerate(pairs):
        ra = res.results[2 * pi]
        rb = res.results[2 * pi + 1]
        # out{j}: [128, ND, w] -> [D, w]; concat chunks -> [D, cap] -> .T
        cols = []
        for j in range(len(chunks)):
            h2j = (np.asarray(ra[f"out{j}"], np.float32)
                   + np.asarray(rb[f"out{j}"], np.float32))
            cols.append(h2j.transpose(1, 0, 2).reshape(D, -1))
        h2 = np.concatenate(cols, axis=1).T     # [cap, D]
        out[idxs[a]] += wts[a][:, None] * h2[:len(idxs[a])]
        out[idxs[b_]] += wts[b_][:, None] * h2[n1g:n1g + len(idxs[b_])]
    return out.reshape(B, L, D).astype(np.float32)


# revision 23
# speedup vs baseline: 1.0087x; 1.0025x over previous
"""Trainium2 Bass kernel for nn_AdaptiveExpertSystem (MoE, E=8, top-2).

Expert-parallel + pairwise tensor-parallel design. The host computes the
(cheap) router on CPU and uses it as the sharding function. Experts are
paired heavy+light (LPT), one pair per pair of cores: both cores of a
pair receive ALL tokens routed to either expert (pre-normalized by the
LN the router already computed, pre-transposed to feature-major), but
each core holds only HALF of the pair's w1/w2 along the intermediate
dimension I. Each core computes partial MLP outputs for all the pair's
tokens at the PE bf16 matmul roofline; the host sums the two halves and
scatter-adds with the top-2 combine weights (the unshard step).

ln_g/ln_b are folded into w1/b1 on the host so all experts share the
plain LN.

v2 (DMA restructure): the PE is the bottleneck (one continuous matmul
block at ~0.505 ns/col); the rest is head/tail engineering.
 - Every token chunk gets its OWN contiguous SBUF tile, so token DMAs
   have multi-KB per-partition runs (fast big-line path) instead of
   strided ~1KB writes into one wide tile (slow ~24 GB/s/queue path).
 - chunk0 and the final chunk are narrowed to 256 columns: chunk0's
   tokens gate the first matmul; the final chunk's output gates the
   kernel end.
 - Outputs are written per-chunk in [128, ND, w] layout and DMAed in
   d-groups of 4 (2 for the last chunk) -> 2-4 KB runs, alternating
   the sync/scalar queues.
 - Bulk weights ride gpsimd in consumption order; token chunks
   round-robin sync/scalar.
"""
import numpy as np
import ml_dtypes

import concourse.bass as bass
import concourse.tile as tile
from concourse import bacc, mybir
from concourse.bass_utils import run_bass_kernel_spmd

N_CORES = 8
B, L, D, I, E = 2, 2048, 1024, 4096, 8
NTOK = B * L
KD = D // 128       # 8  d-tiles (contraction of mm1)
NI = I // 128       # 32 i-tiles total; each core runs NIH = 16
NIH = NI // 2
ND = D // 128       # 8  output d-tiles
LN_EPS = 1e-5

F32 = mybir.dt.float32
BF16 = mybir.dt.bfloat16
BF = ml_dtypes.bfloat16

_CACHE = {}


def _chunks_one(n, first):
    # chunk widths for one expert's n columns; every chunk in [288, 508]
    # (>=288 keeps the ~117ns LDWEIGHTS hidden behind the previous
    # matmul at 0.425 ns/col, <=508 fits a PSUM bank). slot0 leads with
    # a 384 chunk: narrow enough that its token DMA doesn't gate the
    # first matmul for long, wide enough that its weight-tile demand
    # (256KB per 8*w cols) stays under the gpsimd supply rate.
    ws = []
    rest = n
    head = 0
    if first and n > 892:
        head = 384
        rest -= 384
    nch = max(1, (rest + 507) // 508)
    base = rest // nch
    extra = rest - base * nch
    if head:
        ws.append(head)
    for j in range(nch):
        ws.append(base + (1 if j < extra else 0))
    return ws


def _chunk_table(n1, n2):
    """[(expert_slot, col_offset, width), ...] covering [0,n1)+[n1,n1+n2)."""
    out = []
    o = 0
    for sl, n in ((0, n1), (1, n2)):
        for w in _chunks_one(n, first=(sl == 0)):
            out.append((sl, o, w))
            o += w
    return out


def _process_order(chunks):
    """Chunk execution order: start with slot0's head chunk, end with a
    mid-size slot0 chunk (small output tail). Slot1 chunks run in the
    middle-to-late region so their weights (which stream on gpsimd
    after slot0's) arrive in time."""
    s0 = [j for j, (sl, _, _) in enumerate(chunks) if sl == 0]
    s1 = [j for j, (sl, _, _) in enumerate(chunks) if sl == 1]
    if len(s0) >= 2:
        return s0[:-1] + s1 + [s0[-1]]
    return s0 + s1


def build_nc(n1, n2):
    chunks = _chunk_table(n1, n2)

    nc = bacc.Bacc(None, num_devices=N_CORES)
    xt_ps = [nc.declare_dram_parameter(f"xt{j}", [128, KD, w], BF16,
                                       isOutput=False)
             for j, (_, _, w) in enumerate(chunks)]
    # slot0 weights as per-tile arrays (need-ordered individual DMAs
    # during the head crunch); slot1 weights partition-major so each
    # transfers as ONE big DMA (32KB/partition contiguous runs)
    w1_p = nc.declare_dram_parameter("w1", [NIH, 128, KD, 128], BF16,
                                     isOutput=False)
    w2_p = nc.declare_dram_parameter("w2", [ND, 128, NIH, 128], BF16,
                                     isOutput=False)
    w1b_p = nc.declare_dram_parameter("w1b", [128, NIH, KD, 128], BF16,
                                      isOutput=False)
    w2b_p = nc.declare_dram_parameter("w2b", [128, ND, NIH, 128], BF16,
                                      isOutput=False)
    b1_p = nc.declare_dram_parameter("b1", [128, 2, NIH], F32, isOutput=False)
    out_ps = [nc.declare_dram_parameter(f"out{j}", [128, ND, w], BF16,
                                        isOutput=True)
              for j, (_, _, w) in enumerate(chunks)]

    AF = mybir.ActivationFunctionType

    from contextlib import ExitStack
    with tile.TileContext(nc) as tc, ExitStack() as ctx:
        ep = ctx.enter_context
        xtp = ep(tc.tile_pool(name="xtp", bufs=1))
        w1pool = ep(tc.tile_pool(name="w1p", bufs=1))
        w2pool = ep(tc.tile_pool(name="w2p", bufs=1))
        b1pool = ep(tc.tile_pool(name="b1p", bufs=1))
        h1pool = ep(tc.tile_pool(name="h1p", bufs=1))
        h2pool = ep(tc.tile_pool(name="h2p", bufs=2))
        ps1 = ep(tc.tile_pool(name="ps1", bufs=4, space="PSUM"))
        ps2 = ep(tc.tile_pool(name="ps2", bufs=2, space="PSUM"))

        b1sb = b1pool.tile([128, 2, NIH], F32)
        nc.scalar.dma_start(out=b1sb, in_=b1_p[:])
        w1sb = w1pool.tile([128, 2, NIH, KD, 128], BF16)
        w2sb = w2pool.tile([128, 2, ND, NIH, 128], BF16)
        # one contiguous SBUF tile per token chunk: per-partition runs of
        # KD*w*2 bytes make these DMAs ride the fast big-line path
        xsb = [xtp.tile([128, KD, w], BF16, name=f"xsb{j}")
               for j, (_, _, w) in enumerate(chunks)]

        # The scalar (Activation) engine must carry NO bulk DMA issue:
        # each HWDGE ring holds only ~2 outstanding transfers, so a
        # DMA_DIRECT2D on scalar blocks until older transfers drain —
        # which postpones the GELU ACTIVATEs, which stalls the PE on
        # PSUM-bank reuse (matmul i needs activation i-4 done). So:
        # scalar gets only the tiny b1; sync + gpsimd split everything
        # else in need order. The first-matmul gate is chunk0's tokens
        # + w1[0,0]; chunk0's mm1 sweeps all 4MB of w1[slot0] at
        # ~190GB/s demand vs ~122+135 GB/s combined queue rate, so
        # slot0 w1 tiles alternate sync/gpsimd by need index.
        nc.sync.dma_start(out=xsb[0][:, 0:KD // 2],
                          in_=xt_ps[0][:, 0:KD // 2, :])
        nc.gpsimd.dma_start(out=w1sb[:, 0, 0], in_=w1_p[0])
        # chunk0's upper k-half split scalar/gpsimd: scalar can afford
        # exactly ONE early bulk transfer before its ACTIVATEs start
        nc.scalar.dma_start(out=xsb[0][:, KD // 2:KD // 2 + 2],
                            in_=xt_ps[0][:, KD // 2:KD // 2 + 2, :])
        nc.gpsimd.dma_start(out=xsb[0][:, KD // 2 + 2:],
                            in_=xt_ps[0][:, KD // 2 + 2:, :])
        for i in range(1, NIH):
            q = nc.sync if i % 2 else nc.gpsimd
            q.dma_start(out=w1sb[:, 0, i], in_=w1_p[i])
        for d in range(ND):
            q = nc.gpsimd if d % 2 == 0 else nc.sync
            q.dma_start(out=w2sb[:, 0, d], in_=w2_p[d])
        # remaining token chunks on sync in execution order (first
        # needed at ~55us, they queue behind the slot0 weights); slot1
        # weights on gpsimd (consumed only from ~40% through => slack)
        order = _process_order(chunks)
        for j in order[1:]:
            nc.sync.dma_start(out=xsb[j], in_=xt_ps[j][:])
        for i in range(NIH):
            nc.gpsimd.dma_start(out=w1sb[:, 1, i], in_=w1b_p[:, i])
        for d in range(ND):
            nc.gpsimd.dma_start(out=w2sb[:, 1, d], in_=w2b_p[:, d])

        out_qs = [nc.sync, nc.gpsimd]
        _oq = [0]

        def out_dma(dst, src):
            q = out_qs[_oq[0] % 2]
            _oq[0] += 1
            q.dma_start(out=dst, in_=src)

        for oi, ci in enumerate(order):
            sl, c0, w = chunks[ci]
            # ---- mm1 + gelu -> h1 (this chunk, this expert slot) ----
            h1 = h1pool.tile([128, NIH, w], BF16, name=f"h1_{ci}", tag="h1",
                             bufs=1)
            for i in range(NIH):
                p1 = ps1.tile([128, w], F32, tag="p1", name=f"p1_{ci}_{i}")
                for k in range(KD):
                    nc.tensor.matmul(p1, lhsT=w1sb[:, sl, i, k],
                                     rhs=xsb[ci][:, k],
                                     start=(k == 0), stop=(k == KD - 1))
                nc.scalar.activation(out=h1[:, i], in_=p1, func=AF.Gelu,
                                     bias=b1sb[:, sl, i:i + 1], scale=1.0)
            # ---- mm2 -> partial h2 -> grouped DMA out ----
            # d-groups of 4 (2 on the final chunk) give 2-4KB DRAM runs
            last = (oi == len(order) - 1)
            G = 2 if last else 4
            for d0 in range(0, ND, G):
                h2 = h2pool.tile([128, G, w], BF16, tag="h2",
                                 name=f"h2_{ci}_{d0}")
                for dd in range(0, G, 2):
                    # p2 spans TWO psum banks (padded to the 2KB bank
                    # stride) so one CAST covers a d-pair
                    p2 = ps2.tile([128, 2, w], F32, tag="p2",
                                  name=f"p2_{ci}_{d0 + dd}",
                                  padded_shape=[128, 2, 512])
                    for g in range(2):
                        d = d0 + dd + g
                        for i in range(NIH):
                            nc.tensor.matmul(p2[:, g], lhsT=w2sb[:, sl, d, i],
                                             rhs=h1[:, i],
                                             start=(i == 0),
                                             stop=(i == NIH - 1))
                    nc.vector.tensor_copy(out=h2[:, dd:dd + 2], in_=p2)
                out_dma(out_ps[ci][:, d0:d0 + G, :], h2)

    nc.finalize()
    return nc


def _pack_w1h(w1e_half):
    # [d, i_half] -> [i_tile, p, k, m]; d = k*128 + p, i = it*128 + m
    t = w1e_half.reshape(KD, 128, NIH, 128)
    return np.ascontiguousarray(t.transpose(2, 1, 0, 3)).astype(BF)


def _pack_w2h(w2e_half):
    # [i_half, d] -> [d_tile, p, i_tile, m]; i = it*128 + p, d = dt*128 + m
    t = w2e_half.reshape(NIH, 128, ND, 128)
    return np.ascontiguousarray(t.transpose(2, 1, 0, 3)).astype(BF)


def kernel(**inputs) -> np.ndarray:
    x = np.asarray(inputs["hidden_states"], np.float32).reshape(NTOK, D)
    rn_g = np.asarray(inputs["rn_g"], np.float32)
    rn_b = np.asarray(inputs["rn_b"], np.float32)
    router_w = np.asarray(inputs["router_w"], np.float32)
    router_b = np.asarray(inputs["router_b"], np.float32)
    ln_g = np.asarray(inputs["ln_g"], np.float32)
    ln_b = np.asarray(inputs["ln_b"], np.float32)
    w1 = np.asarray(inputs["w1"], np.float32)
    b1 = np.asarray(inputs["b1"], np.float32)
    w2 = np.asarray(inputs["w2"], np.float32)
    b2 = np.asarray(inputs["b2"], np.float32)

    # ---- Router on host: this IS the sharding function ----
    m = x.mean(-1, keepdims=True)
    v = ((x - m) ** 2).mean(-1, keepdims=True)
    rstd = 1.0 / np.sqrt(v + LN_EPS)
    normed = (x - m) * rstd
    logits = (normed * rn_g + rn_b) @ router_w.T + router_b
    top2 = np.argsort(-logits, axis=-1, kind="stable")[:, :2]
    tv = np.take_along_axis(logits, top2, -1)
    tv = np.exp(tv - tv.max(-1, keepdims=True))
    tw = (tv / tv.sum(-1, keepdims=True)).astype(np.float32)

    idxs, wts = [], []
    for e in range(E):
        sel = (top2[:, 0] == e) | (top2[:, 1] == e)
        idx_e = np.nonzero(sel)[0]
        w_e = np.where(top2[idx_e, 0] == e, tw[idx_e, 0], tw[idx_e, 1])
        idxs.append(idx_e)
        wts.append(w_e.astype(np.float32))

    # LPT pairing: heaviest with lightest
    order = sorted(range(E), key=lambda e: -len(idxs[e]))
    pairs = [(order[i], order[E - 1 - i]) for i in range(E // 2)]
    # round counts to multiples of 4 for clean strides
    caps = {e: max(256, ((len(idxs[e]) + 3) // 4) * 4) for e in range(E)}
    n1g = max(caps[a] for a, b in pairs)
    n2g = max(caps[b] for a, b in pairs)

    normed_bf = normed.astype(BF)
    in_maps = []
    chunks = _chunk_table(n1g, n2g)
    for a, b_ in pairs:
        g = np.zeros((n1g + n2g, D), dtype=BF)
        g[:len(idxs[a])] = normed_bf[idxs[a]]
        g[n1g:n1g + len(idxs[b_])] = normed_bf[idxs[b_]]
        t = g.T.reshape(KD, 128, n1g + n2g).transpose(1, 0, 2)
        xts = {}
        for j, (sl, c0, w) in enumerate(chunks):
            xts[f"xt{j}"] = np.ascontiguousarray(t[:, :, c0:c0 + w])
        for half in range(2):
            ilo, ihi = half * (I // 2), (half + 1) * (I // 2)
            w1sl = [_pack_w1h((ln_g[e][:, None] * w1[e])[:, ilo:ihi])
                    for e in (a, b_)]
            w2sl = [_pack_w2h(w2[e][ilo:ihi, :]) for e in (a, b_)]
            b1s = np.stack(
                [(b1[e] + ln_b[e] @ w1[e])[ilo:ihi].reshape(NIH, 128).T
                 .astype(np.float32) for e in (a, b_)], axis=1)
            mp = {"w1": w1sl[0], "w2": w2sl[0],
                  "w1b": np.ascontiguousarray(w1sl[1].transpose(1, 0, 2, 3)),
                  "w2b": np.ascontiguousarray(w2sl[1].transpose(1, 0, 2, 3)),
                  "b1": np.ascontiguousarray(b1s)}
            mp.update(xts)
            in_maps.append(mp)

    key = (n1g, n2g)
    if key not in _CACHE:
        _CACHE[key] = build_nc(n1g, n2g)
    nc = _CACHE[key]
    res = run_bass_kernel_spmd(nc, in_maps, core_ids=list(range(N_CORES)))

    # ---- Unshard: sum I-halves, weighted scatter-add (top-2 combine) ----
    out = tw[:, 0:1] * b2[top2[:, 0]] + tw[:, 1:2] * b2[top2[:, 1]]
    for pi, (a, b_) in enumerate(pairs):
        ra = res.results[2 * pi]
        rb = res.results[2 * pi + 1]
        # out{j}: [128, ND, w] -> [D, w]; concat chunks -> [D, cap] -> .T
        cols = []
        for j in range(len(chunks)):
            h2j = (np.asarray(ra[f"out{j}"], np.float32)
                   + np.asarray(rb[f"out{j}"], np.float32))
            cols.append(h2j.transpose(1, 0, 2).reshape(D, -1))
        h2 = np.concatenate(cols, axis=1).T     # [cap, D]
        out[idxs[a]] += wts[a][:, None] * h2[:len(idxs[a])]
        out[idxs[b_]] += wts[b_][:, None] * h2[n1g:n1g + len(idxs[b_])]
    return out.reshape(B, L, D).astype(np.float32)


# revision 26
# speedup vs baseline: 1.0182x; 1.0093x over previous
"""Trainium2 Bass kernel for nn_AdaptiveExpertSystem (MoE, E=8, top-2).

Expert-parallel + pairwise tensor-parallel design. The host computes the
(cheap) router on CPU and uses it as the sharding function. Experts are
paired heavy+light (LPT), one pair per pair of cores: both cores of a
pair receive ALL tokens routed to either expert (pre-normalized by the
LN the router already computed, pre-transposed to feature-major), but
each core holds only HALF of the pair's w1/w2 along the intermediate
dimension I. Each core computes partial MLP outputs for all the pair's
tokens at the PE bf16 matmul roofline; the host sums the two halves and
scatter-adds with the top-2 combine weights (the unshard step).

ln_g/ln_b are folded into w1/b1 on the host so all experts share the
plain LN.

v2 (DMA restructure): the PE is the bottleneck (one continuous matmul
block at ~0.505 ns/col); the rest is head/tail engineering.
 - Every token chunk gets its OWN contiguous SBUF tile, so token DMAs
   have multi-KB per-partition runs (fast big-line path) instead of
   strided ~1KB writes into one wide tile (slow ~24 GB/s/queue path).
 - chunk0 and the final chunk are narrowed to 256 columns: chunk0's
   tokens gate the first matmul; the final chunk's output gates the
   kernel end.
 - Outputs are written per-chunk in [128, ND, w] layout and DMAed in
   d-groups of 4 (2 for the last chunk) -> 2-4 KB runs, alternating
   the sync/scalar queues.
 - Bulk weights ride gpsimd in consumption order; token chunks
   round-robin sync/scalar.
"""
import numpy as np
import ml_dtypes

import concourse.bass as bass
import concourse.tile as tile
from concourse import bacc, mybir
from concourse.bass_utils import run_bass_kernel_spmd

N_CORES = 8
B, L, D, I, E = 2, 2048, 1024, 4096, 8
NTOK = B * L
KD = D // 128       # 8  d-tiles (contraction of mm1)
NI = I // 128       # 32 i-tiles total; each core runs NIH = 16
NIH = NI // 2
ND = D // 128       # 8  output d-tiles
LN_EPS = 1e-5

F32 = mybir.dt.float32
BF16 = mybir.dt.bfloat16
BF = ml_dtypes.bfloat16

_CACHE = {}


def _chunks_one(n, first):
    # chunk widths for one expert's n columns; every chunk in [288, 508]
    # (>=288 keeps the ~117ns LDWEIGHTS hidden behind the previous
    # matmul at 0.425 ns/col, <=508 fits a PSUM bank). slot0 leads with
    # a 384 chunk: narrow enough that its token DMA doesn't gate the
    # first matmul for long, wide enough that its weight-tile demand
    # (256KB per 8*w cols) stays under the gpsimd supply rate.
    ws = []
    rest = n
    head = 0
    if first and n > 860:
        head = 352
        rest -= 352
    nch = max(1, (rest + 507) // 508)
    base = rest // nch
    extra = rest - base * nch
    if head:
        ws.append(head)
    for j in range(nch):
        ws.append(base + (1 if j < extra else 0))
    return ws


def _chunk_table(n1, n2):
    """[(expert_slot, col_offset, width), ...] covering [0,n1)+[n1,n1+n2)."""
    out = []
    o = 0
    for sl, n in ((0, n1), (1, n2)):
        for w in _chunks_one(n, first=(sl == 0)):
            out.append((sl, o, w))
            o += w
    return out


def _process_order(chunks):
    """Chunk execution order: start with slot0's head chunk, end with a
    mid-size slot0 chunk (small output tail). Slot1 chunks run in the
    middle-to-late region so their weights (which stream on gpsimd
    after slot0's) arrive in time."""
    s0 = [j for j, (sl, _, _) in enumerate(chunks) if sl == 0]
    s1 = [j for j, (sl, _, _) in enumerate(chunks) if sl == 1]
    if len(s0) >= 2:
        return s0[:-1] + s1 + [s0[-1]]
    return s0 + s1


def build_nc(n1, n2):
    chunks = _chunk_table(n1, n2)

    nc = bacc.Bacc(None, num_devices=N_CORES)
    xt_ps = [nc.declare_dram_parameter(f"xt{j}", [128, KD, w], BF16,
                                       isOutput=False)
             for j, (_, _, w) in enumerate(chunks)]
    # slot0 weights as per-tile arrays (need-ordered individual DMAs
    # during the head crunch); slot1 weights partition-major so each
    # transfers as ONE big DMA (32KB/partition contiguous runs)
    w1_p = nc.declare_dram_parameter("w1", [NIH, 128, KD, 128], BF16,
                                     isOutput=False)
    w2_p = nc.declare_dram_parameter("w2", [ND, 128, NIH, 128], BF16,
                                     isOutput=False)
    w1b_p = nc.declare_dram_parameter("w1b", [128, NIH, KD, 128], BF16,
                                      isOutput=False)
    w2b_p = nc.declare_dram_parameter("w2b", [128, ND, NIH, 128], BF16,
                                      isOutput=False)
    b1_p = nc.declare_dram_parameter("b1", [128, 2, NIH], F32, isOutput=False)
    out_ps = [nc.declare_dram_parameter(f"out{j}", [128, ND, w], BF16,
                                        isOutput=True)
              for j, (_, _, w) in enumerate(chunks)]

    AF = mybir.ActivationFunctionType

    from contextlib import ExitStack
    with tile.TileContext(nc) as tc, ExitStack() as ctx:
        ep = ctx.enter_context
        xtp = ep(tc.tile_pool(name="xtp", bufs=1))
        w1pool = ep(tc.tile_pool(name="w1p", bufs=1))
        w2pool = ep(tc.tile_pool(name="w2p", bufs=1))
        b1pool = ep(tc.tile_pool(name="b1p", bufs=1))
        h1pool = ep(tc.tile_pool(name="h1p", bufs=1))
        h2pool = ep(tc.tile_pool(name="h2p", bufs=2))
        ps1 = ep(tc.tile_pool(name="ps1", bufs=4, space="PSUM"))
        ps2 = ep(tc.tile_pool(name="ps2", bufs=2, space="PSUM"))

        b1sb = b1pool.tile([128, 2, NIH], F32)
        nc.scalar.dma_start(out=b1sb, in_=b1_p[:])
        w1sb = w1pool.tile([128, 2, NIH, KD, 128], BF16)
        w2sb = w2pool.tile([128, 2, ND, NIH, 128], BF16)
        # one contiguous SBUF tile per token chunk: per-partition runs of
        # KD*w*2 bytes make these DMAs ride the fast big-line path
        xsb = [xtp.tile([128, KD, w], BF16, name=f"xsb{j}")
               for j, (_, _, w) in enumerate(chunks)]

        # The scalar (Activation) engine must carry NO bulk DMA issue:
        # each HWDGE ring holds only ~2 outstanding transfers, so a
        # DMA_DIRECT2D on scalar blocks until older transfers drain —
        # which postpones the GELU ACTIVATEs, which stalls the PE on
        # PSUM-bank reuse (matmul i needs activation i-4 done). So:
        # scalar gets only the tiny b1; sync + gpsimd split everything
        # else in need order. The first-matmul gate is chunk0's tokens
        # + w1[0,0]; chunk0's mm1 sweeps all 4MB of w1[slot0] at
        # ~190GB/s demand vs ~122+135 GB/s combined queue rate, so
        # slot0 w1 tiles alternate sync/gpsimd by need index.
        nc.sync.dma_start(out=xsb[0][:, 0:KD // 2],
                          in_=xt_ps[0][:, 0:KD // 2, :])
        nc.gpsimd.dma_start(out=w1sb[:, 0, 0], in_=w1_p[0])
        # chunk0's upper k-half split scalar/gpsimd: scalar can afford
        # exactly ONE early bulk transfer before its ACTIVATEs start
        nc.scalar.dma_start(out=xsb[0][:, KD // 2:KD // 2 + 2],
                            in_=xt_ps[0][:, KD // 2:KD // 2 + 2, :])
        nc.gpsimd.dma_start(out=xsb[0][:, KD // 2 + 2:],
                            in_=xt_ps[0][:, KD // 2 + 2:, :])
        for i in range(1, NIH):
            q = nc.sync if i % 2 else nc.gpsimd
            q.dma_start(out=w1sb[:, 0, i], in_=w1_p[i])
        for d in range(ND):
            q = nc.gpsimd if d % 2 == 0 else nc.sync
            q.dma_start(out=w2sb[:, 0, d], in_=w2_p[d])
        # remaining token chunks on sync in execution order (first
        # needed at ~55us, they queue behind the slot0 weights); slot1
        # weights on gpsimd (consumed only from ~40% through => slack)
        order = _process_order(chunks)
        for j in order[1:]:
            nc.sync.dma_start(out=xsb[j], in_=xt_ps[j][:])
        for i in range(NIH):
            nc.gpsimd.dma_start(out=w1sb[:, 1, i], in_=w1b_p[:, i])
        for d in range(ND):
            nc.gpsimd.dma_start(out=w2sb[:, 1, d], in_=w2b_p[:, d])

        out_qs = [nc.sync, nc.gpsimd]
        _oq = [0]

        def out_dma(dst, src, force_sync=False):
            q = nc.sync if force_sync else out_qs[_oq[0] % 2]
            _oq[0] += 1
            q.dma_start(out=dst, in_=src)

        for oi, ci in enumerate(order):
            sl, c0, w = chunks[ci]
            # ---- mm1 + gelu -> h1 (this chunk, this expert slot) ----
            h1 = h1pool.tile([128, NIH, w], BF16, name=f"h1_{ci}", tag="h1",
                             bufs=1)
            for i in range(NIH):
                p1 = ps1.tile([128, w], F32, tag="p1", name=f"p1_{ci}_{i}")
                for k in range(KD):
                    nc.tensor.matmul(p1, lhsT=w1sb[:, sl, i, k],
                                     rhs=xsb[ci][:, k],
                                     start=(k == 0), stop=(k == KD - 1))
                nc.scalar.activation(out=h1[:, i], in_=p1, func=AF.Gelu,
                                     bias=b1sb[:, sl, i:i + 1], scale=1.0)
            # ---- mm2 -> partial h2 -> grouped DMA out ----
            # d-groups of 4 (2 on the final chunk) give 2-4KB DRAM runs
            last = (oi == len(order) - 1)
            G = 2 if last else 4
            for d0 in range(0, ND, G):
                h2 = h2pool.tile([128, G, w], BF16, tag="h2",
                                 name=f"h2_{ci}_{d0}")
                for dd in range(0, G, 2):
                    # p2 spans TWO psum banks (padded to the 2KB bank
                    # stride) so one CAST covers a d-pair
                    p2 = ps2.tile([128, 2, w], F32, tag="p2",
                                  name=f"p2_{ci}_{d0 + dd}",
                                  padded_shape=[128, 2, 512])
                    for g in range(2):
                        d = d0 + dd + g
                        for i in range(NIH):
                            nc.tensor.matmul(p2[:, g], lhsT=w2sb[:, sl, d, i],
                                             rhs=h1[:, i],
                                             start=(i == 0),
                                             stop=(i == NIH - 1))
                    nc.vector.tensor_copy(out=h2[:, dd:dd + 2], in_=p2)
                out_dma(out_ps[ci][:, d0:d0 + G, :], h2, force_sync=last)

    nc.finalize()
    return nc


def _pack_w1h(w1e_half):
    # [d, i_half] -> [i_tile, p, k, m]; d = k*128 + p, i = it*128 + m
    t = w1e_half.reshape(KD, 128, NIH, 128)
    return np.ascontiguousarray(t.transpose(2, 1, 0, 3)).astype(BF)


def _pack_w2h(w2e_half):
    # [i_half, d] -> [d_tile, p, i_tile, m]; i = it*128 + p, d = dt*128 + m
    t = w2e_half.reshape(NIH, 128, ND, 128)
    return np.ascontiguousarray(t.transpose(2, 1, 0, 3)).astype(BF)


def kernel(**inputs) -> np.ndarray:
    x = np.asarray(inputs["hidden_states"], np.float32).reshape(NTOK, D)
    rn_g = np.asarray(inputs["rn_g"], np.float32)
    rn_b = np.asarray(inputs["rn_b"], np.float32)
    router_w = np.asarray(inputs["router_w"], np.float32)
    router_b = np.asarray(inputs["router_b"], np.float32)
    ln_g = np.asarray(inputs["ln_g"], np.float32)
    ln_b = np.asarray(inputs["ln_b"], np.float32)
    w1 = np.asarray(inputs["w1"], np.float32)
    b1 = np.asarray(inputs["b1"], np.float32)
    w2 = np.asarray(inputs["w2"], np.float32)
    b2 = np.asarray(inputs["b2"], np.float32)

    # ---- Router on host: this IS the sharding function ----
    m = x.mean(-1, keepdims=True)
    v = ((x - m) ** 2).mean(-1, keepdims=True)
    rstd = 1.0 / np.sqrt(v + LN_EPS)
    normed = (x - m) * rstd
    logits = (normed * rn_g + rn_b) @ router_w.T + router_b
    top2 = np.argsort(-logits, axis=-1, kind="stable")[:, :2]
    tv = np.take_along_axis(logits, top2, -1)
    tv = np.exp(tv - tv.max(-1, keepdims=True))
    tw = (tv / tv.sum(-1, keepdims=True)).astype(np.float32)

    idxs, wts = [], []
    for e in range(E):
        sel = (top2[:, 0] == e) | (top2[:, 1] == e)
        idx_e = np.nonzero(sel)[0]
        w_e = np.where(top2[idx_e, 0] == e, tw[idx_e, 0], tw[idx_e, 1])
        idxs.append(idx_e)
        wts.append(w_e.astype(np.float32))

    # LPT pairing: heaviest with lightest
    order = sorted(range(E), key=lambda e: -len(idxs[e]))
    pairs = [(order[i], order[E - 1 - i]) for i in range(E // 2)]
    # round counts to multiples of 4 for clean strides
    caps = {e: max(256, ((len(idxs[e]) + 3) // 4) * 4) for e in range(E)}
    n1g = max(caps[a] for a, b in pairs)
    n2g = max(caps[b] for a, b in pairs)

    normed_bf = normed.astype(BF)
    in_maps = []
    chunks = _chunk_table(n1g, n2g)
    for a, b_ in pairs:
        g = np.zeros((n1g + n2g, D), dtype=BF)
        g[:len(idxs[a])] = normed_bf[idxs[a]]
        g[n1g:n1g + len(idxs[b_])] = normed_bf[idxs[b_]]
        t = g.T.reshape(KD, 128, n1g + n2g).transpose(1, 0, 2)
        xts = {}
        for j, (sl, c0, w) in enumerate(chunks):
            xts[f"xt{j}"] = np.ascontiguousarray(t[:, :, c0:c0 + w])
        for half in range(2):
            ilo, ihi = half * (I // 2), (half + 1) * (I // 2)
            w1sl = [_pack_w1h((ln_g[e][:, None] * w1[e])[:, ilo:ihi])
                    for e in (a, b_)]
            w2sl = [_pack_w2h(w2[e][ilo:ihi, :]) for e in (a, b_)]
            b1s = np.stack(
                [(b1[e] + ln_b[e] @ w1[e])[ilo:ihi].reshape(NIH, 128).T
                 .astype(np.float32) for e in (a, b_)], axis=1)
            mp = {"w1": w1sl[0], "w2": w2sl[0],
                  "w1b": np.ascontiguousarray(w1sl[1].transpose(1, 0, 2, 3)),
                  "w2b": np.ascontiguousarray(w2sl[1].transpose(1, 0, 2, 3)),
                  "b1": np.ascontiguousarray(b1s)}
            mp.update(xts)
            in_maps.append(mp)

    key = (n1g, n2g)
    if key not in _CACHE:
        _CACHE[key] = build_nc(n1g, n2g)
    nc = _CACHE[key]
    res = run_bass_kernel_spmd(nc, in_maps, core_ids=list(range(N_CORES)))

    # ---- Unshard: sum I-halves, weighted scatter-add (top-2 combine) ----
    out = tw[:, 0:1] * b2[top2[:, 0]] + tw[:, 1:2] * b2[top2[:, 1]]
    for pi, (a, b_) in enumerate(pairs):
        ra = res.results[2 * pi]
        rb = res.results[2 * pi + 1]
        # out{j}: [128, ND, w] -> [D, w]; concat chunks -> [D, cap] -> .T
        cols = []
        for j in range(len(chunks)):
            h2j = (np.asarray(ra[f"out{j}"], np.float32)
                   + np.asarray(rb[f"out{j}"], np.float32))
            cols.append(h2j.transpose(1, 0, 2).reshape(D, -1))
        h2 = np.concatenate(cols, axis=1).T     # [cap, D]
        out[idxs[a]] += wts[a][:, None] * h2[:len(idxs[a])]
        out[idxs[b_]] += wts[b_][:, None] * h2[n1g:n1g + len(idxs[b_])]
    return out.reshape(B, L, D).astype(np.float32)
